# revision 1
# baseline (speedup 1.0000x reference)
"""Trainium2 Bass kernel for DeepOdoModel (CNN feature extractor + GRU recurrence).

Strategy:
- Data-parallel over batch: B=16 -> 2 batch elements per core across 8 cores.
  Each core is fully independent (no collectives).
- All activations are kept feature-major ("transposed", features on SBUF
  partitions) through the whole network so the GRU elementwise gate math runs
  on 128 partitions and the recurrent state never needs transposing.
- conv1 via im2col (single 4D-AP DMA per frame chunk, K=77), conv2 via
  shift-accumulation in PSUM (K=128, 9 shifts), fc1+fc2 folded into one
  512x1536 matmul (host-side weight fusion), GRU input projection (gi)
  precomputed for all timesteps, biases all folded host-side.
- GRU: 512 sequential steps; per step 48 matmuls (whh.T stationary tiles,
  h.T moving [128,2]) producing feature-major gates, then 8 DVE/ACT ops.
"""

import sys

if "/opt/trn_rl_repo" not in sys.path:
    sys.path.insert(0, "/opt/trn_rl_repo")

import numpy as np

B, T_FULL, L, C = 16, 512, 50, 7
H = 512
NCORES = 8
BL = B // NCORES  # 2


def build_nc(T=T_FULL, num_devices=NCORES, debug=False, col_tile=False,
             whh_bf16=False):
    import concourse.bass as bass
    import concourse.mybir as mybir
    import concourse.tile as tile
    from concourse import bacc
    from concourse.alu_op_type import AluOpType

    f32 = mybir.dt.float32
    AF = mybir.ActivationFunctionType
    NF = BL * T
    F1 = 8
    F2 = 32
    F3 = min(128, NF)
    n3 = NF // F3
    n2 = F3 // F2
    n1 = F2 // F1

    nc = bacc.Bacc("TRN2", target_bir_lowering=False, debug=debug,
                   num_devices=num_devices)

    xim = nc.dram_tensor("xim", [77, NF * 40], f32, kind="ExternalInput")
    h0t = nc.dram_tensor("h0t", [128, 8], f32, kind="ExternalInput")
    w1 = nc.dram_tensor("w1", [77, 128], f32, kind="ExternalInput")
    w2t = nc.dram_tensor("w2t", [128, 9 * 256], f32, kind="ExternalInput")
    wct = nc.dram_tensor("wct", [12 * 128, 512], f32, kind="ExternalInput")
    wiht = nc.dram_tensor("wiht", [512, 1536], f32, kind="ExternalInput")
    bf16 = mybir.dt.bfloat16
    whh_dt = bf16 if whh_bf16 else f32
    whht = nc.dram_tensor("whht", [512, 1536], whh_dt, kind="ExternalInput")
    if whh_bf16:
        h0tb = nc.dram_tensor("h0tb", [128, 8], bf16, kind="ExternalInput")
    gib = nc.dram_tensor("gib", [128, 12], f32, kind="ExternalInput")
    bhhn = nc.dram_tensor("bhhn", [128, 4], f32, kind="ExternalInput")
    fc3w = nc.dram_tensor("fc3w", [128, 4], f32, kind="ExternalInput")
    fc3b = nc.dram_tensor("fc3b", [1, 1], f32, kind="ExternalInput")
    out = nc.dram_tensor("out", [1, 2 * T], f32, kind="ExternalOutput")

    with tile.TileContext(nc) as tc:
        with tc.tile_pool(name="weights", bufs=1) as wp:
            w1sb = wp.tile([77, 128], f32)
            nc.sync.dma_start(out=w1sb, in_=w1.ap())
            w2sb = wp.tile([128, 9 * 256], f32)
            nc.sync.dma_start(out=w2sb, in_=w2t.ap())
            wcsb = wp.tile([128, 12 * 512], f32)
            for kt in range(12):
                nc.sync.dma_start(out=wcsb[:, kt * 512:(kt + 1) * 512],
                                  in_=wct.ap()[kt * 128:(kt + 1) * 128, :])
            wihsb = wp.tile([128, 4 * 1536], f32)
            whhsb = wp.tile([128, 4 * 1536], whh_dt)
            for k in range(4):
                nc.sync.dma_start(out=wihsb[:, k * 1536:(k + 1) * 1536],
                                  in_=wiht.ap()[k * 128:(k + 1) * 128, :])
                nc.sync.dma_start(out=whhsb[:, k * 1536:(k + 1) * 1536],
                                  in_=whht.ap()[k * 128:(k + 1) * 128, :])
            gibsb = wp.tile([128, 12], f32)
            nc.sync.dma_start(out=gibsb, in_=gib.ap())
            bhhnsb = wp.tile([128, 4], f32)
            nc.sync.dma_start(out=bhhnsb, in_=bhhn.ap())
            fc3wsb = wp.tile([128, 4], f32)
            nc.sync.dma_start(out=fc3wsb, in_=fc3w.ap())
            fc3bsb = wp.tile([1, 1], f32)
            nc.sync.dma_start(out=fc3bsb, in_=fc3b.ap())
            h0sb = wp.tile([128, 8], f32)
            nc.sync.dma_start(out=h0sb, in_=h0t.ap())
            if whh_bf16:
                h0sbb = wp.tile([128, 8], bf16)
                nc.sync.dma_start(out=h0sbb, in_=h0tb.ap())

            # persistent activations
            giT = wp.tile([128, 12 * NF], f32)   # (m, b, t) feature-major gi
            hsT = wp.tile([128, T * 8], f32)     # (t, k, b) hidden states
            if whh_bf16:
                hsTb = wp.tile([128, T * 8], bf16)  # bf16 copy for matmul rhs

            # ---------------- CNN + FC + gi ----------------
            with tc.tile_pool(name="ps_cnn", bufs=6, space="PSUM") as psp, \
                 tc.tile_pool(name="xb", bufs=3) as xpool, \
                 tc.tile_pool(name="p1", bufs=2) as p1pool, \
                 tc.tile_pool(name="p2", bufs=2) as p2pool, \
                 tc.tile_pool(name="ft", bufs=2) as ftpool:
                for b3 in range(n3):
                    p2t = p2pool.tile([128, 2 * F3 * 6], f32)
                    for b2 in range(n2):
                        p1t = p1pool.tile([128, F2 * 20], f32)
                        for c1 in range(n1):
                            n0 = b3 * F3 + b2 * F2 + c1 * F1
                            x1 = xpool.tile([77, F1 * 40], f32)
                            nc.sync.dma_start(
                                out=x1,
                                in_=xim.ap()[:, n0 * 40:(n0 + F1) * 40])
                            ps1 = psp.tile([128, F1 * 40], f32, tag="ps")
                            nc.tensor.matmul(
                                ps1[:],
                                lhsT=w1sb[:],
                                rhs=x1,
                                start=True, stop=True,
                            )
                            nc.vector.tensor_reduce(
                                out=p1t[:, c1 * F1 * 20:(c1 + 1) * F1 * 20],
                                in_=ps1.rearrange("p (a two) -> p a two", two=2),
                                axis=mybir.AxisListType.X, op=AluOpType.max,
                            )
                        # conv2 over this 32-frame block
                        p1v = p1t.rearrange("p (n l) -> p n l", l=20)
                        for m in range(2):
                            ps2 = psp.tile([128, F2 * 12], f32, tag="ps")
                            for k in range(9):
                                nc.tensor.matmul(
                                    ps2[:],
                                    lhsT=w2sb[:, k * 256 + m * 128:
                                              k * 256 + m * 128 + 128],
                                    rhs=p1v[:, :, k:k + 12],
                                    start=(k == 0), stop=(k == 8),
                                )
                            nc.vector.tensor_reduce(
                                out=p2t[:, m * F3 * 6 + b2 * F2 * 6:
                                        m * F3 * 6 + (b2 + 1) * F2 * 6],
                                in_=ps2.rearrange("p (a two) -> p a two", two=2),
                                axis=mybir.AxisListType.X, op=AluOpType.max,
                            )
                    # fused fc1*fc2 -> featT
                    ft = ftpool.tile([128, 4 * F3], f32)
                    p2v = p2t.rearrange("p (c n l) -> p c n l", c=2, l=6)
                    for m4 in range(4):
                        ps3 = psp.tile([128, F3], f32, tag="ps")
                        for kt in range(12):
                            p_, cm = kt // 2, kt % 2
                            nc.tensor.matmul(
                                ps3[:],
                                lhsT=wcsb[:, kt * 512 + m4 * 128:
                                          kt * 512 + m4 * 128 + 128],
                                rhs=p2v[:, cm, :, p_:p_ + 1],
                                start=(kt == 0), stop=(kt == 11),
                            )
                        nc.scalar.copy(ft[:, m4 * F3:(m4 + 1) * F3], ps3[:])
                    # gi projection -> giT
                    for m in range(12):
                        ps4 = psp.tile([128, F3], f32, tag="ps")
                        for k in range(4):
                            nc.tensor.matmul(
                                ps4[:],
                                lhsT=wihsb[:, k * 1536 + m * 128:
                                           k * 1536 + m * 128 + 128],
                                rhs=ft[:, k * F3:(k + 1) * F3],
                                start=(k == 0), stop=(k == 3),
                            )
                        nc.scalar.activation(
                            giT[:, m * NF + b3 * F3:m * NF + (b3 + 1) * F3],
                            ps4[:], AF.Identity, bias=gibsb[:, m:m + 1])

            # ---------------- GRU recurrence ----------------
            giv = giT.rearrange("p (m b tt) -> p m b tt", m=12, b=BL)
            MORDER = [0, 1, 2, 3, 8, 9, 10, 11, 4, 5, 6, 7]
            with tc.tile_pool(name="psg", bufs=2, space="PSUM") as psgp, \
                 tc.tile_pool(name="gt", bufs=3) as gtp:
                from concourse.tile_rust import add_dep_helper
                for t in range(T):
                    hprev = h0sb if t == 0 else hsT[:, (t - 1) * 8:t * 8]
                    if whh_bf16:
                        hprev_mm = h0sbb if t == 0 else hsTb[:, (t - 1) * 8:t * 8]
                    else:
                        hprev_mm = hprev
                    psg = psgp.tile([128, 24], f32)
                    # PE instructions must stay in emission order: accumulation
                    # groups share a PSUM bank and start=True clears has_written
                    # bank-wide, so interleaving groups corrupts partial sums.
                    prev_mm = None
                    for m in MORDER:
                        for k in range(4):
                            if col_tile:
                                for cg in range(4):
                                    base = k * 1536 + m * 128 + 32 * cg
                                    mm = nc.tensor.matmul(
                                        psg[32 * cg:32 * cg + 32,
                                            2 * m:2 * m + 2],
                                        lhsT=whhsb[:, base:base + 32],
                                        rhs=hprev_mm[:, 2 * k:2 * k + 2],
                                        start=(k == 0), stop=(k == 3),
                                        tile_position=(0, 32 * cg),
                                    )
                                    if prev_mm is not None:
                                        add_dep_helper(
                                            mm.ins, prev_mm.ins,
                                            reason="psum group order")
                                    prev_mm = mm
                            else:
                                mm = nc.tensor.matmul(
                                    psg[:, 2 * m:2 * m + 2],
                                    lhsT=whhsb[:, k * 1536 + m * 128:
                                               k * 1536 + m * 128 + 128],
                                    rhs=hprev_mm[:, 2 * k:2 * k + 2],
                                    start=(k == 0), stop=(k == 3),
                                )
                                if prev_mm is not None:
                                    add_dep_helper(mm.ins, prev_mm.ins,
                                                   reason="psum group order")
                                prev_mm = mm
                    # r gate (ready first: m 0-3)
                    rp = gtp.tile([128, 4, 2], f32)
                    nc.vector.tensor_tensor(
                        out=rp,
                        in0=psg[:, 0:8].rearrange("p (m b) -> p m b", b=2),
                        in1=giv[:, 0:4, :, t], op=AluOpType.add)
                    rt = gtp.tile([128, 4, 2], f32)
                    nc.scalar.activation(rt, rp, AF.Sigmoid)
                    # n gate (m 8-11): n = tanh(gi_n + r*(hn + bhh_n))
                    tmp = gtp.tile([128, 4, 2], f32)
                    for j in range(4):
                        nc.vector.scalar_tensor_tensor(
                            out=tmp[:, j:j + 1, :],
                            in0=psg[:, 16 + 2 * j:18 + 2 * j].rearrange(
                                "p (o b) -> p o b", o=1),
                            scalar=bhhnsb[:, j:j + 1],
                            in1=rt[:, j:j + 1, :],
                            op0=AluOpType.add, op1=AluOpType.mult)
                    npre = gtp.tile([128, 4, 2], f32)
                    nc.vector.tensor_tensor(out=npre, in0=tmp,
                                            in1=giv[:, 8:12, :, t],
                                            op=AluOpType.add)
                    nt = gtp.tile([128, 4, 2], f32)
                    nc.scalar.activation(nt, npre, AF.Tanh)
                    hp3 = hprev.rearrange("p (k b) -> p k b", b=2)
                    d = gtp.tile([128, 4, 2], f32)
                    nc.vector.tensor_tensor(out=d, in0=hp3, in1=nt,
                                            op=AluOpType.subtract)
                    # z gate (ready last: m 4-7); h = n + z*(h - n)
                    zp = gtp.tile([128, 4, 2], f32)
                    nc.vector.tensor_tensor(
                        out=zp,
                        in0=psg[:, 8:16].rearrange("p (m b) -> p m b", b=2),
                        in1=giv[:, 4:8, :, t], op=AluOpType.add)
                    zt = gtp.tile([128, 4, 2], f32)
                    nc.scalar.activation(zt, zp, AF.Sigmoid)
                    e = gtp.tile([128, 4, 2], f32)
                    nc.vector.tensor_tensor(out=e, in0=d, in1=zt,
                                            op=AluOpType.mult)
                    hnew = hsT[:, t * 8:(t + 1) * 8].rearrange(
                        "p (k b) -> p k b", b=2)
                    nc.vector.tensor_tensor(out=hnew, in0=e, in1=nt,
                                            op=AluOpType.add)
                    if whh_bf16:
                        nc.vector.tensor_copy(
                            hsTb[:, t * 8:(t + 1) * 8],
                            hsT[:, t * 8:(t + 1) * 8])

            # ---------------- output head ----------------
            with tc.tile_pool(name="pso", bufs=2, space="PSUM") as psop, \
                 tc.tile_pool(name="ot", bufs=1) as otp:
                osb = otp.tile([1, 2 * T], f32)
                hs4 = hsT.rearrange("p (tt k b) -> p tt k b", k=4, b=2)
                tc_chunk = min(256, T)
                for ch in range(T // tc_chunk):
                    pso = psop.tile([1, tc_chunk, 2], f32)
                    for k in range(4):
                        nc.tensor.matmul(
                            pso[:],
                            lhsT=fc3wsb[:, k:k + 1],
                            rhs=hs4[:, ch * tc_chunk:(ch + 1) * tc_chunk, k, :],
                            start=(k == 0), stop=(k == 3),
                        )
                    nc.scalar.activation(
                        osb[:, ch * tc_chunk * 2:(ch + 1) * tc_chunk * 2]
                        .rearrange("p (tt b) -> p tt b", b=2),
                        pso[:], AF.Identity, bias=fc3bsb[:, 0:1])
                nc.sync.dma_start(out=out.ap(), in_=osb)

    nc.compile()
    return nc


def prep_inputs(inputs, T=T_FULL, whh_bf16=False):
    """Host-side weight folding + per-core input shards."""
    f = np.float32
    conv1_w = inputs["conv1_w"].astype(f)  # [128, 7, 11]
    conv1_b = inputs["conv1_b"].astype(f)
    conv2_w = inputs["conv2_w"].astype(f)  # [256, 128, 9]
    conv2_b = inputs["conv2_b"].astype(f)
    fc1_w = inputs["fc1_w"].astype(f)      # [1024, 1536]
    fc1_b = inputs["fc1_b"].astype(f)
    fc2_w = inputs["fc2_w"].astype(f)      # [512, 1024]
    fc2_b = inputs["fc2_b"].astype(f)
    wih = inputs["gru_wih"].astype(f)      # [1536, 512]
    whh = inputs["gru_whh"].astype(f)
    bih = inputs["gru_bih"].astype(f)
    bhh = inputs["gru_bhh"].astype(f)
    fc3_w = inputs["fc3_w"].astype(f)      # [1, 512]
    fc3_b = inputs["fc3_b"].astype(f)

    w1 = np.ascontiguousarray(
        conv1_w.transpose(1, 2, 0).reshape(77, 128))  # (c,k) rows
    w2t = np.ascontiguousarray(
        conv2_w.transpose(1, 2, 0).reshape(128, 9 * 256))  # [i, (k,o)]

    Wc = fc2_w @ fc1_w                      # [512, 1536]
    b2_eff = conv2_b + np.einsum("oik,i->o", conv2_w, conv1_b)
    b2_flat = np.repeat(b2_eff, 6)          # [1536] channel-major flatten
    bc_eff = fc2_w @ fc1_b + fc2_b + Wc @ b2_flat  # [512]

    WcT = Wc.T                              # [1536, 512]
    wct = np.empty((12 * 128, 512), f)
    for p in range(6):
        for cm in range(2):
            kt = p * 2 + cm
            rows = 6 * (cm * 128 + np.arange(128)) + p
            wct[kt * 128:(kt + 1) * 128] = WcT[rows]

    gi_bias = bih + wih @ bc_eff
    gi_bias[:1024] += bhh[:1024]            # fold bhh for r,z gates
    gib = np.ascontiguousarray(gi_bias.reshape(12, 128).T)   # [128, 12]
    bhhn = np.ascontiguousarray(bhh[1024:].reshape(4, 128).T)  # [128, 4]
    fc3wt = np.ascontiguousarray(fc3_w[0].reshape(4, 128).T)   # [128, 4]

    wihT = np.ascontiguousarray(wih.T)      # [512, 1536]
    whhT = np.ascontiguousarray(whh.T)
    if whh_bf16:
        import ml_dtypes
        whhT = whhT.astype(ml_dtypes.bfloat16)

    phone = inputs["phone_data"].astype(f)  # [B, T, L, C]
    h0 = inputs["h0"].astype(f)             # [B, H]

    in_maps = []
    for c in range(NCORES):
        psh = phone[c * BL:(c + 1) * BL]    # [2, T, 50, 7]
        NF = BL * T
        xsh2 = psh.reshape(NF, L, C)        # [NF, 50, 7]
        xim = np.empty((77, NF, 40), f)
        for cc in range(C):
            for k in range(11):
                xim[cc * 11 + k] = xsh2[:, k:k + 40, cc]
        xim = xim.reshape(77, NF * 40)
        h0sh = h0[c * BL:(c + 1) * BL]      # [2, 512]
        h0tt = np.ascontiguousarray(
            h0sh.reshape(BL, 4, 128).transpose(2, 1, 0).reshape(128, 8))
        in_maps.append({
            "xim": xim, "h0t": h0tt, "w1": w1, "w2t": w2t, "wct": wct,
            "wiht": wihT, "whht": whhT, "gib": gib, "bhhn": bhhn,
            "fc3w": fc3wt, "fc3b": fc3_b.reshape(1, 1).astype(f),
        })
    return in_maps


def assemble_output(results, T=T_FULL):
    full = np.empty((B, T, 1), np.float32)
    for c in range(NCORES):
        o = results[c]["out"].reshape(T, BL)  # cols (t, b)
        full[c * BL:(c + 1) * BL, :, 0] = o.T
    return full


_NC_CACHE = {}


def kernel(**inputs):
    from concourse import bass_utils

    if "nc" not in _NC_CACHE:
        _NC_CACHE["nc"] = build_nc()
    nc = _NC_CACHE["nc"]
    in_maps = prep_inputs(inputs)
    res = bass_utils.run_bass_kernel_spmd(nc, in_maps,
                                          core_ids=list(range(NCORES)))
    return assemble_output(res.results)



# revision 3
# speedup vs baseline: 2.4148x; 2.4148x over previous
"""Trainium2 Bass kernel for DeepOdoModel (CNN feature extractor + GRU).

Single-core design (per-core NEFF launches serialize through this PJRT
path, so total device time is minimized by putting all 16 batch lanes on
one core):
- CNN/FC/gi stage in bf16 (moving operands bf16 -> 4x PE throughput),
  fp32 PSUM accumulation, frame order t-major so gi streams contiguously.
- gi projections spilled to DRAM (too big for SBUF at BL=16) and streamed
  back during the GRU in 32-step chunks (double buffered).
- GRU: biases/gi_rz folded into PSUM via identity-weight matmuls, r-gate
  matmuls emitted first so the sigmoid starts early; hidden state kept
  bf16 (matmul rhs) with fp32 gate math.
"""

import sys

if "/opt/trn_rl_repo" not in sys.path:
    sys.path.insert(0, "/opt/trn_rl_repo")

import numpy as np

B, T_FULL, L, C = 16, 512, 50, 7
H = 512
NCORES = 1
BL = B  # all batch lanes on one core


def build_nc(T=T_FULL, debug=False):
    import concourse.mybir as mybir
    import concourse.tile as tile
    from concourse import bacc
    from concourse.alu_op_type import AluOpType
    from concourse.tile_rust import add_dep_helper

    f32 = mybir.dt.float32
    bf16 = mybir.dt.bfloat16
    AF = mybir.ActivationFunctionType
    NF = BL * T
    F1 = 8
    F2 = 32
    F3 = 128
    n3 = NF // F3          # 64 blocks of 128 frames (8 t-steps x 16 b)
    n2 = F3 // F2
    n1 = F2 // F1
    TB = F3 // BL          # t-steps per block = 8
    NCH = 32               # GRU steps per gi stream chunk
    BPC = NCH // TB        # CNN blocks per gi chunk = 4

    nc = bacc.Bacc("TRN2", target_bir_lowering=False, debug=debug,
                   num_devices=NCORES)

    xim = nc.dram_tensor("xim", [77, NF * 40], bf16, kind="ExternalInput")
    h0t = nc.dram_tensor("h0t", [128, BL * 4], bf16, kind="ExternalInput")
    w1 = nc.dram_tensor("w1", [77, 128], bf16, kind="ExternalInput")
    w2t = nc.dram_tensor("w2t", [128, 9 * 256], bf16, kind="ExternalInput")
    wct = nc.dram_tensor("wct", [12 * 128, 512], bf16, kind="ExternalInput")
    wiht = nc.dram_tensor("wiht", [512, 1536], bf16, kind="ExternalInput")
    whht = nc.dram_tensor("whht", [512, 1536], bf16, kind="ExternalInput")
    ident = nc.dram_tensor("ident", [128, 128], bf16, kind="ExternalInput")
    bhhb = nc.dram_tensor("bhhb", [128, 4 * BL], bf16, kind="ExternalInput")
    gib = nc.dram_tensor("gib", [128, 12], f32, kind="ExternalInput")
    fc3w = nc.dram_tensor("fc3w", [128, 4], bf16, kind="ExternalInput")
    fc3b = nc.dram_tensor("fc3b", [1, 1], f32, kind="ExternalInput")
    giRZ = nc.dram_tensor("giRZ", [128, T * 8 * BL], bf16,
                          kind="ExternalInput")
    giN = nc.dram_tensor("giN", [128, T * 4 * BL], f32, kind="ExternalInput")
    out = nc.dram_tensor("out", [1, NF], f32, kind="ExternalOutput")

    GRZ = 8 * BL   # rz cols per step (j,b) = 128
    GN = 4 * BL    # n cols per step = 64
    HC = 4 * BL    # h cols per step (k,b) = 64

    with tile.TileContext(nc) as tc:
        with tc.tile_pool(name="weights", bufs=1) as wp:
            w1sb = wp.tile([77, 128], bf16)
            nc.sync.dma_start(out=w1sb, in_=w1.ap())
            w2sb = wp.tile([128, 9 * 256], bf16)
            nc.sync.dma_start(out=w2sb, in_=w2t.ap())
            wcsb = wp.tile([128, 12 * 512], bf16)
            for kt in range(12):
                nc.sync.dma_start(out=wcsb[:, kt * 512:(kt + 1) * 512],
                                  in_=wct.ap()[kt * 128:(kt + 1) * 128, :])
            wihsb = wp.tile([128, 4 * 1536], bf16)
            whhsb = wp.tile([128, 4 * 1536], bf16)
            for k in range(4):
                nc.sync.dma_start(out=wihsb[:, k * 1536:(k + 1) * 1536],
                                  in_=wiht.ap()[k * 128:(k + 1) * 128, :])
                nc.sync.dma_start(out=whhsb[:, k * 1536:(k + 1) * 1536],
                                  in_=whht.ap()[k * 128:(k + 1) * 128, :])
            idsb = wp.tile([128, 128], bf16)
            nc.sync.dma_start(out=idsb, in_=ident.ap())
            bhhsb = wp.tile([128, 4 * BL], bf16)
            nc.sync.dma_start(out=bhhsb, in_=bhhb.ap())
            gibsb = wp.tile([128, 12], f32)
            nc.sync.dma_start(out=gibsb, in_=gib.ap())
            fc3wsb = wp.tile([128, 4], bf16)
            nc.sync.dma_start(out=fc3wsb, in_=fc3w.ap())
            fc3bsb = wp.tile([1, 1], f32)
            nc.sync.dma_start(out=fc3bsb, in_=fc3b.ap())
            h0sb = wp.tile([128, HC], bf16)
            nc.sync.dma_start(out=h0sb, in_=h0t.ap())

            # persistent hidden states (bf16, feeds both GRU matmuls + head)
            hsT = wp.tile([128, T * HC], bf16)

            rz_out_dmas = []
            n_out_dmas = []

            # ---------------- CNN + FC + gi (spilled to DRAM) ------------
            with tc.tile_pool(name="ps_cnn", bufs=6, space="PSUM") as psp, \
                 tc.tile_pool(name="xb", bufs=3) as xpool, \
                 tc.tile_pool(name="p1", bufs=2) as p1pool, \
                 tc.tile_pool(name="p2", bufs=2) as p2pool, \
                 tc.tile_pool(name="ft", bufs=2) as ftpool, \
                 tc.tile_pool(name="gst", bufs=2) as gspool:
                for b3 in range(n3):
                    p2t = p2pool.tile([128, 2 * F3 * 6], bf16)
                    for b2 in range(n2):
                        p1t = p1pool.tile([128, F2 * 20], bf16)
                        for c1 in range(n1):
                            n0 = b3 * F3 + b2 * F2 + c1 * F1
                            x1 = xpool.tile([77, F1 * 40], bf16)
                            nc.sync.dma_start(
                                out=x1,
                                in_=xim.ap()[:, n0 * 40:(n0 + F1) * 40])
                            ps1 = psp.tile([128, F1 * 40], f32, tag="ps")
                            nc.tensor.matmul(
                                ps1[:], lhsT=w1sb[:], rhs=x1,
                                start=True, stop=True)
                            nc.vector.tensor_reduce(
                                out=p1t[:, c1 * F1 * 20:(c1 + 1) * F1 * 20],
                                in_=ps1.rearrange("p (a two) -> p a two",
                                                  two=2),
                                axis=mybir.AxisListType.X, op=AluOpType.max)
                        p1v = p1t.rearrange("p (n l) -> p n l", l=20)
                        for m in range(2):
                            ps2 = psp.tile([128, F2 * 12], f32, tag="ps")
                            for k in range(9):
                                nc.tensor.matmul(
                                    ps2[:],
                                    lhsT=w2sb[:, k * 256 + m * 128:
                                              k * 256 + m * 128 + 128],
                                    rhs=p1v[:, :, k:k + 12],
                                    start=(k == 0), stop=(k == 8))
                            nc.vector.tensor_reduce(
                                out=p2t[:, m * F3 * 6 + b2 * F2 * 6:
                                        m * F3 * 6 + (b2 + 1) * F2 * 6],
                                in_=ps2.rearrange("p (a two) -> p a two",
                                                  two=2),
                                axis=mybir.AxisListType.X, op=AluOpType.max)
                    ft = ftpool.tile([128, 4 * F3], bf16)
                    p2v = p2t.rearrange("p (c n l) -> p c n l", c=2, l=6)
                    for m4 in range(4):
                        ps3 = psp.tile([128, F3], f32, tag="ps")
                        for kt in range(12):
                            p_, cm = kt // 2, kt % 2
                            nc.tensor.matmul(
                                ps3[:],
                                lhsT=wcsb[:, kt * 512 + m4 * 128:
                                          kt * 512 + m4 * 128 + 128],
                                rhs=p2v[:, cm, :, p_:p_ + 1],
                                start=(kt == 0), stop=(kt == 11))
                        nc.scalar.copy(ft[:, m4 * F3:(m4 + 1) * F3], ps3[:])
                    # gi projections -> staging tiles -> DRAM
                    gRZt = gspool.tile([128, TB * GRZ], bf16, tag="grz")
                    gNt = gspool.tile([128, TB * GN], f32, tag="gn")
                    gRZv = gRZt.rearrange("p (tt c) -> p tt c", c=GRZ)
                    gNv = gNt.rearrange("p (tt c) -> p tt c", c=GN)
                    for j in range(12):
                        ps4 = psp.tile([128, F3], f32, tag="ps")
                        for k in range(4):
                            nc.tensor.matmul(
                                ps4[:],
                                lhsT=wihsb[:, k * 1536 + j * 128:
                                           k * 1536 + j * 128 + 128],
                                rhs=ft[:, k * F3:(k + 1) * F3],
                                start=(k == 0), stop=(k == 3))
                        src = ps4.rearrange("p (tt b) -> p tt b", b=BL)
                        if j < 8:
                            dst = gRZv[:, :, j * BL:(j + 1) * BL]
                        else:
                            dst = gNv[:, :, (j - 8) * BL:(j - 7) * BL]
                        nc.scalar.activation(dst, src, AF.Identity,
                                             bias=gibsb[:, j:j + 1])
                    d1 = nc.sync.dma_start(
                        out=giRZ.ap()[:, b3 * TB * GRZ:(b3 + 1) * TB * GRZ],
                        in_=gRZt)
                    d2 = nc.sync.dma_start(
                        out=giN.ap()[:, b3 * TB * GN:(b3 + 1) * TB * GN],
                        in_=gNt)
                    rz_out_dmas.append(d1)
                    n_out_dmas.append(d2)

            # ---------------- GRU recurrence -----------------------------
            with tc.tile_pool(name="psa", bufs=2, space="PSUM") as psap, \
                 tc.tile_pool(name="psb", bufs=2, space="PSUM") as psbp, \
                 tc.tile_pool(name="gin", bufs=2) as ginp, \
                 tc.tile_pool(name="gt", bufs=3) as gtp:
                prev_mm = None
                for ch in range(T // NCH):
                    grz_t = ginp.tile([128, NCH * GRZ], bf16, tag="rz")
                    gn_t = ginp.tile([128, NCH * GN], f32, tag="n")
                    di1 = nc.sync.dma_start(
                        out=grz_t,
                        in_=giRZ.ap()[:, ch * NCH * GRZ:(ch + 1) * NCH * GRZ])
                    di2 = nc.sync.dma_start(
                        out=gn_t,
                        in_=giN.ap()[:, ch * NCH * GN:(ch + 1) * NCH * GN])
                    blk = ch * BPC + BPC - 1
                    add_dep_helper(di1.ins, rz_out_dmas[blk].ins,
                                   reason="gi dram raw")
                    add_dep_helper(di2.ins, n_out_dmas[blk].ins,
                                   reason="gi dram raw")
                    for tl in range(NCH):
                        t = ch * NCH + tl
                        hprev = h0sb if t == 0 else hsT[:, (t - 1) * HC:
                                                        t * HC]
                        psgA = psap.tile([128, GRZ], f32)
                        psgB = psbp.tile([128, GN], f32)
                        # PSUM accumulation: one start=True per region;
                        # strict PE emission order via dep chain.
                        mms = []
                        mms.append(nc.tensor.matmul(
                            psgA[:], lhsT=idsb[:],
                            rhs=grz_t[:, tl * GRZ:(tl + 1) * GRZ],
                            start=True, stop=False, skip_group_check=True))
                        for m in range(8):
                            for k in range(4):
                                mms.append(nc.tensor.matmul(
                                    psgA[:, m * BL:(m + 1) * BL],
                                    lhsT=whhsb[:, k * 1536 + m * 128:
                                               k * 1536 + m * 128 + 128],
                                    rhs=hprev[:, k * BL:(k + 1) * BL],
                                    start=False, stop=(k == 3),
                                    skip_group_check=True))
                        mms.append(nc.tensor.matmul(
                            psgB[:], lhsT=idsb[:], rhs=bhhsb[:],
                            start=True, stop=False, skip_group_check=True))
                        for m in range(4):
                            for k in range(4):
                                mms.append(nc.tensor.matmul(
                                    psgB[:, m * BL:(m + 1) * BL],
                                    lhsT=whhsb[:, k * 1536 + (m + 8) * 128:
                                               k * 1536 + (m + 8) * 128 + 128],
                                    rhs=hprev[:, k * BL:(k + 1) * BL],
                                    start=False, stop=(k == 3),
                                    skip_group_check=True))
                        for mm in mms:
                            if prev_mm is not None:
                                add_dep_helper(mm.ins, prev_mm.ins,
                                               reason="psum group order")
                            prev_mm = mm
                        rt = gtp.tile([128, 4 * BL], f32)
                        nc.scalar.activation(rt, psgA[:, 0:4 * BL],
                                             AF.Sigmoid)
                        zt = gtp.tile([128, 4 * BL], f32)
                        nc.scalar.activation(zt, psgA[:, 4 * BL:8 * BL],
                                             AF.Sigmoid)
                        tmp = gtp.tile([128, GN], f32)
                        nc.vector.tensor_tensor(out=tmp, in0=psgB[:],
                                                in1=rt[:],
                                                op=AluOpType.mult)
                        npre = gtp.tile([128, GN], f32)
                        nc.vector.tensor_tensor(
                            out=npre, in0=tmp,
                            in1=gn_t[:, tl * GN:(tl + 1) * GN],
                            op=AluOpType.add)
                        nt = gtp.tile([128, GN], f32)
                        nc.scalar.activation(nt, npre, AF.Tanh)
                        d = gtp.tile([128, GN], f32)
                        nc.vector.tensor_tensor(out=d, in0=hprev, in1=nt,
                                                op=AluOpType.subtract)
                        e = gtp.tile([128, GN], f32)
                        nc.vector.tensor_tensor(out=e, in0=d, in1=zt,
                                                op=AluOpType.mult)
                        nc.vector.tensor_tensor(
                            out=hsT[:, t * HC:(t + 1) * HC], in0=e, in1=nt,
                            op=AluOpType.add)

            # ---------------- output head --------------------------------
            with tc.tile_pool(name="pso", bufs=2, space="PSUM") as psop, \
                 tc.tile_pool(name="ot", bufs=1) as otp:
                osb = otp.tile([1, NF], f32)
                hs4 = hsT.rearrange("p (tt k b) -> p tt k b", k=4, b=BL)
                CH = 32
                for ch in range(T // CH):
                    pso = psop.tile([1, CH * BL], f32)
                    for k in range(4):
                        nc.tensor.matmul(
                            pso[:],
                            lhsT=fc3wsb[:, k:k + 1],
                            rhs=hs4[:, ch * CH:(ch + 1) * CH, k, :],
                            start=(k == 0), stop=(k == 3))
                    nc.scalar.activation(
                        osb[:, ch * CH * BL:(ch + 1) * CH * BL],
                        pso[:], AF.Identity, bias=fc3bsb[:, 0:1])
                nc.sync.dma_start(out=out.ap(), in_=osb)

    nc.compile()
    return nc


def prep_inputs(inputs, T=T_FULL):
    import ml_dtypes
    bf = ml_dtypes.bfloat16
    f = np.float32
    conv1_w = inputs["conv1_w"].astype(f)
    conv1_b = inputs["conv1_b"].astype(f)
    conv2_w = inputs["conv2_w"].astype(f)
    conv2_b = inputs["conv2_b"].astype(f)
    fc1_w = inputs["fc1_w"].astype(f)
    fc1_b = inputs["fc1_b"].astype(f)
    fc2_w = inputs["fc2_w"].astype(f)
    fc2_b = inputs["fc2_b"].astype(f)
    wih = inputs["gru_wih"].astype(f)
    whh = inputs["gru_whh"].astype(f)
    bih = inputs["gru_bih"].astype(f)
    bhh = inputs["gru_bhh"].astype(f)
    fc3_w = inputs["fc3_w"].astype(f)
    fc3_b = inputs["fc3_b"].astype(f)

    w1 = np.ascontiguousarray(
        conv1_w.transpose(1, 2, 0).reshape(77, 128)).astype(bf)
    w2tt = np.ascontiguousarray(
        conv2_w.transpose(1, 2, 0).reshape(128, 9 * 256)).astype(bf)

    Wc = fc2_w @ fc1_w
    b2_eff = conv2_b + np.einsum("oik,i->o", conv2_w, conv1_b)
    b2_flat = np.repeat(b2_eff, 6)
    bc_eff = fc2_w @ fc1_b + fc2_b + Wc @ b2_flat

    WcT = Wc.T
    wct = np.empty((12 * 128, 512), f)
    for p in range(6):
        for cm in range(2):
            kt = p * 2 + cm
            rows = 6 * (cm * 128 + np.arange(128)) + p
            wct[kt * 128:(kt + 1) * 128] = WcT[rows]
    wct = wct.astype(bf)

    gi_bias = bih + wih @ bc_eff
    gi_bias[:1024] += bhh[:1024]
    gib = np.ascontiguousarray(gi_bias.reshape(12, 128).T).astype(f)
    bhhn = bhh[1024:].reshape(4, 128).T            # [128, 4]
    bhhb = np.repeat(bhhn[:, :, None], BL, axis=2).reshape(128, 4 * BL)
    bhhb = np.ascontiguousarray(bhhb).astype(bf)
    fc3wt = np.ascontiguousarray(fc3_w[0].reshape(4, 128).T).astype(bf)

    wihT = np.ascontiguousarray(wih.T).astype(bf)
    whhT = np.ascontiguousarray(whh.T).astype(bf)
    ident = np.eye(128, dtype=f).astype(bf)

    phone = inputs["phone_data"].astype(f)         # [B, T, L, C]
    h0 = inputs["h0"].astype(f)

    NF = BL * T
    xt = np.ascontiguousarray(
        phone.transpose(1, 0, 2, 3).reshape(NF, L, C))  # t-major frames
    sw = np.lib.stride_tricks.sliding_window_view(xt, 40, axis=1)
    # sw: [NF, 11, C, 40] with sw[f, k, c, j] = xt[f, k + j, c]
    xim = np.ascontiguousarray(
        sw.transpose(2, 1, 0, 3)).reshape(77, NF * 40).astype(bf)

    h0tt = np.ascontiguousarray(
        h0.reshape(BL, 4, 128).transpose(2, 1, 0).reshape(128, 4 * BL))
    h0tt = h0tt.astype(bf)

    giRZ = np.zeros((128, T * 8 * BL), bf)
    giN = np.zeros((128, T * 4 * BL), f)

    in_map = {
        "xim": xim, "h0t": h0tt, "w1": w1, "w2t": w2tt, "wct": wct,
        "wiht": wihT, "whht": whhT, "ident": ident, "bhhb": bhhb,
        "gib": gib, "fc3w": fc3wt,
        "fc3b": fc3_b.reshape(1, 1).astype(f),
        "giRZ": giRZ, "giN": giN,
    }
    return [in_map]


def assemble_output(results, T=T_FULL):
    o = results[0]["out"].reshape(T, BL)   # col = t*BL + b
    full = np.ascontiguousarray(o.T).reshape(BL, T, 1).astype(np.float32)
    return full


_NC_CACHE = {}


def kernel(**inputs):
    from concourse import bass_utils

    if "nc" not in _NC_CACHE:
        _NC_CACHE["nc"] = build_nc()
    nc = _NC_CACHE["nc"]
    in_maps = prep_inputs(inputs)
    res = bass_utils.run_bass_kernel_spmd(nc, in_maps, core_ids=[0])
    return assemble_output(res.results)
